# revision 10
# baseline (speedup 1.0000x reference)
"""Trainium2 kernel for nn_BaselineRelationalIndependentModel:
out = sigmoid(W2d[x, y]) with W2d = W.reshape(2048, 2048), B = 16,777,216.

Sharding: data-parallel — batch split evenly across the 8 NeuronCores; the
16 MiB weight table is replicated (each core reads it from its own HBM).

Device kernel (per core, 2,097,152 lookups):
  1. flat = 2048*x + y on VectorE (int32 shift/or) over [128, C] tiles.
  2. Batched gather via ONE gpsimd indirect DMA per R = 128*C lookups:
     the offset AP is the whole [128, C] tile and the dest is a
     single-partition row [1, R, 1].  The SWDGE dynamic-DMA ucode emits one
     4-byte descriptor per offset, consuming offsets partition-cycling
     (desc k <- offs[k % 128, k // 128]) and writing dest element k — so a
     host-side column-major element layout makes the gathered row exactly
     the element stream.  This amortizes the ~1 us fixed SWDGE cost over
     8192 lookups instead of 128 (the previous kernel's per-column calls
     cost 16384 calls * ~1.4 us = 23 ms; measured drain rate now is
     ~5-6 ns/lookup, bounded by the single dynamic-DMA queue).
  3. sigmoid on ScalarE directly on the [1, R] row; DMA the row out.

Host reshapes to column-major per core before the run and reassembles the
[B, 1] output afterwards.
"""

import numpy as np

import concourse.bass as bass
import concourse.bacc as bacc
import concourse.mybir as mybir
import concourse.tile as tile
from concourse.bass_utils import run_bass_kernel_spmd

NOBJ = 2048
TAB = NOBJ * NOBJ          # 4,194,304 table entries
B = 16777216
NCORES = 8
BPC = B // NCORES          # 2,097,152 lookups per core
P = 128
F = BPC // P               # 16384 columns per core
CB = 64                    # columns per gather call -> R = 8192 lookups
R = P * CB


def build_nc(f_total: int = F, cb: int = CB) -> bacc.Bacc:
    # Default 2 MiB descriptor carveout: a 4 MiB carveout measured faster on
    # one core (5.77 vs 6.15 ns/lookup) but slower on the full 8-core run
    # (15.2 vs 12.9 ms), so keep the default.
    nc = bacc.Bacc(None, target_bir_lowering=False)
    xd = nc.dram_tensor("x", [P, f_total], mybir.dt.int32, kind="ExternalInput")
    yd = nc.dram_tensor("y", [P, f_total], mybir.dt.int32, kind="ExternalInput")
    wd = nc.dram_tensor("w", [TAB, 1], mybir.dt.float32, kind="ExternalInput")
    r = P * cb
    nblocks = (f_total + cb - 1) // cb
    od = nc.dram_tensor("out", [nblocks, r], mybir.dt.float32, kind="ExternalOutput")

    with tile.TileContext(nc) as tc:
        with (
            tc.tile_pool(name="io", bufs=6) as io,
            tc.tile_pool(name="val", bufs=3) as vp,
            tc.tile_pool(name="res", bufs=3) as rp,
        ):
            for blk in range(nblocks):
                c0 = blk * cb
                xb = io.tile([P, cb], mybir.dt.int32, tag="xb")
                yb = io.tile([P, cb], mybir.dt.int32, tag="yb")
                nc.sync.dma_start(out=xb[:], in_=xd[:, c0:c0 + cb])
                nc.sync.dma_start(out=yb[:], in_=yd[:, c0:c0 + cb])

                flat = io.tile([P, cb], mybir.dt.int32, tag="flat")
                nc.vector.tensor_scalar(
                    out=flat[:], in0=xb[:], scalar1=11, scalar2=None,
                    op0=mybir.AluOpType.logical_shift_left,
                )
                nc.vector.tensor_tensor(
                    out=flat[:], in0=flat[:], in1=yb[:],
                    op=mybir.AluOpType.bitwise_or,
                )

                val = vp.tile([1, r], mybir.dt.float32, tag="val")
                offs = flat[:].bitcast(mybir.dt.uint32)
                nc.gpsimd.indirect_dma_start(
                    out=val[:].unsqueeze(2),      # [1, R, 1] single-row dest
                    out_offset=None,
                    in_=wd[:],
                    in_offset=bass.IndirectOffsetOnAxis(ap=offs, axis=0),
                )

                res = rp.tile([1, r], mybir.dt.float32, tag="res")
                nc.scalar.activation(
                    out=res[:], in_=val[:],
                    func=mybir.ActivationFunctionType.Sigmoid,
                )
                nc.sync.dma_start(out=od[blk:blk + 1, :], in_=res[:])
    nc.compile()
    return nc


# Set by test harnesses to capture an NTFF profile; the graded path leaves
# this False (no tracing dependencies).
TRACE = False
LAST_EXEC_NS = None

_nc_cache: dict[tuple, bacc.Bacc] = {}


def _get_nc(f_total: int = F, cb: int = CB) -> bacc.Bacc:
    key = (f_total, cb)
    if key not in _nc_cache:
        _nc_cache[key] = build_nc(f_total, cb)
    return _nc_cache[key]


def kernel(x: np.ndarray, y: np.ndarray, W: np.ndarray) -> np.ndarray:
    assert x.shape == (B,) and y.shape == (B,)
    # column-major per core: element k of core c's stream sits at
    # [k % 128, k // 128] so that the gather's partition-cycling offset
    # consumption matches stream order.
    x32 = np.asarray(x).astype(np.int32, copy=False).reshape(NCORES, F, P)
    y32 = np.asarray(y).astype(np.int32, copy=False).reshape(NCORES, F, P)
    x32 = np.ascontiguousarray(x32.transpose(0, 2, 1))
    y32 = np.ascontiguousarray(y32.transpose(0, 2, 1))
    w = np.ascontiguousarray(np.asarray(W, dtype=np.float32).reshape(TAB, 1))

    nc = _get_nc()
    in_maps = [{"x": x32[c], "y": y32[c], "w": w} for c in range(NCORES)]
    res = run_bass_kernel_spmd(
        nc, in_maps, core_ids=list(range(NCORES)), trace=TRACE
    )
    global LAST_EXEC_NS
    LAST_EXEC_NS = res.exec_time_ns
    # out[blk, k] = element blk*R + k of the core's stream
    out = np.concatenate([res.results[c]["out"].reshape(BPC) for c in range(NCORES)])
    return out[:, None]

